# revision 4
# baseline (speedup 1.0000x reference)
"""GCN v4.1: gather-free dense scatter in fp8 with DoubleRow matmuls.

Each core owns 2560 dst slots (identity slot map). The aggregation is
agg = hT @ A where A is the [20480 src, 2560 dst] edge-count matrix,
streamed from DRAM as fp8e4m3 (counts are small ints -> exact) in 80
src-pair-chunks of [128, 2, 2560]. h streams as fp8 [128, 2, 128] lhsT
pair-chunks (host pre-interleaved, no gather). DoubleRow perf mode
contracts 256 srcs per matmul at 0.5 cyc/col. Per-dst 1/deg scaling
happens on-device (DVE multiply by a host-replicated recip row) so fp8
carries no rounding error on the scatter weights.
"""

import numpy as np

N_NODES = 20000
D = 128
N_CORES = 8
N_PAD = 20480
NPC = N_PAD // N_CORES             # 2560 dst slots per core
NCH = N_PAD // 256                 # 80 src pair-chunks
TILE2 = 512
TPT = NPC // TILE2                 # 5 epilogue tiles per core

_prog_cache = {}


def _build_program41():
    import concourse.mybir as mybir
    from concourse import bacc
    from concourse.tile import TileContext

    dt = mybir.dt
    nc = bacc.Bacc()

    h8 = nc.declare_dram_parameter("h8", [NCH, 128, 256], dt.float8e4, isOutput=False)
    smat = nc.declare_dram_parameter(
        "smat", [128, NCH * 2 * NPC], dt.float8e4, isOutput=False
    )
    hT = nc.declare_dram_parameter("hT", [D, NPC], dt.float16, isOutput=False)
    R = nc.declare_dram_parameter("R", [D, NPC], dt.float16, isOutput=False)
    wselfT = nc.declare_dram_parameter("wselfT", [D, D], dt.float16, isOutput=False)
    wneiT = nc.declare_dram_parameter("wneiT", [D, D], dt.float16, isOutput=False)
    bself = nc.declare_dram_parameter("bself", [D, 1], dt.float32, isOutput=False)
    outT = nc.declare_dram_parameter("outT", [D, NPC], dt.float16, isOutput=True)

    with (
        TileContext(nc) as tc,
        tc.tile_pool(name="const", bufs=1) as cpool,
        tc.tile_pool(name="hch", bufs=8) as hpool,
        tc.tile_pool(name="sch", bufs=8) as spool,
        tc.tile_pool(name="agg", bufs=2) as apool,
        tc.tile_pool(name="res", bufs=2) as opool,
        tc.tile_pool(name="pagg", bufs=1, space="PSUM") as pagg,
        tc.tile_pool(name="pout", bufs=2, space="PSUM") as pout,
    ):
        hT_sb = cpool.tile([D, NPC], dt.float16)
        nc.sync.dma_start(out=hT_sb[:], in_=hT[:])
        R_sb = cpool.tile([D, NPC], dt.float16)
        nc.sync.dma_start(out=R_sb[:], in_=R[:])
        wselfT_sb = cpool.tile([D, D], dt.float16)
        nc.sync.dma_start(out=wselfT_sb[:], in_=wselfT[:])
        wneiT_sb = cpool.tile([D, D], dt.float16)
        nc.sync.dma_start(out=wneiT_sb[:], in_=wneiT[:])
        bself_sb = cpool.tile([D, 1], dt.float32)
        nc.sync.dma_start(out=bself_sb[:], in_=bself[:])

        pa = pagg.tile([128, NPC], dt.float32)
        for c in range(NCH):
            hch = hpool.tile([128, 256], dt.float8e4)
            nc.sync.dma_start(out=hch[:], in_=h8[c])
            sch = spool.tile([128, 2 * NPC], dt.float8e4)
            nc.sync.dma_start(
                out=sch[:], in_=smat[:, c * 2 * NPC : (c + 1) * 2 * NPC]
            )
            lhsT = hch[:].rearrange("s (j f) -> s j f", f=128)
            rhs3 = sch[:].rearrange("p (j n) -> p j n", j=2)
            for k in range(TPT):
                nc.tensor.matmul(
                    out=pa[:, k * TILE2 : (k + 1) * TILE2],
                    lhsT=lhsT,
                    rhs=rhs3[:, :, k * TILE2 : (k + 1) * TILE2],
                    perf_mode=mybir.MatmulPerfMode.DoubleRow,
                    start=(c == 0),
                    stop=(c == NCH - 1),
                )

        for k in range(TPT):
            sl = slice(k * TILE2, (k + 1) * TILE2)
            aggT = apool.tile([128, TILE2], dt.float16)
            nc.vector.tensor_mul(out=aggT[:], in0=pa[:, sl], in1=R_sb[:, sl])
            po = pout.tile([128, TILE2], dt.float32)
            nc.tensor.matmul(
                out=po[:], lhsT=wselfT_sb[:], rhs=hT_sb[:, sl], start=True, stop=False
            )
            nc.tensor.matmul(
                out=po[:], lhsT=wneiT_sb[:], rhs=aggT[:], start=False, stop=True
            )
            o = opool.tile([128, TILE2], dt.float16)
            nc.scalar.activation(
                out=o[:],
                in_=po[:],
                func=mybir.ActivationFunctionType.Relu,
                bias=bself_sb[:, :1],
            )
            nc.sync.dma_start(out=outT[:, sl], in_=o[:])

    nc.compile()
    return nc


def kernel(h, edge_index, deg, w_self, b_self, w_nei):
    import os

    import ml_dtypes
    from concourse.bass_utils import run_bass_kernel_spmd

    h = np.asarray(h, dtype=np.float32)
    deg = np.asarray(deg, dtype=np.float32)
    src = np.asarray(edge_index[0], dtype=np.int64)
    dst = np.asarray(edge_index[1], dtype=np.int64)

    # fp8 byte LUT for small integer counts (exact in e4m3)
    lut = np.arange(64, dtype=np.float32).astype(ml_dtypes.float8_e4m3).view(np.uint8)

    hpad = np.zeros((N_PAD, D), dtype=np.float32)
    hpad[:N_NODES] = h
    h8 = hpad.astype(ml_dtypes.float8_e4m3)
    # interleave for DoubleRow lhsT: [pair c, partition s, (ktile j, feat)]
    h8i = np.ascontiguousarray(
        h8.reshape(NCH, 2, 128, D).transpose(0, 2, 1, 3).reshape(NCH, 128, 256)
    )
    hT16 = np.ascontiguousarray(hpad.astype(np.float16).T)  # [128, 20480]
    recip = np.ones(N_PAD, dtype=np.float32)
    recip[:N_NODES] = 1.0 / np.maximum(deg, 1.0)
    R16 = np.ascontiguousarray(
        np.broadcast_to(recip.astype(np.float16)[None, :], (D, N_PAD))
    )
    wselfT = np.ascontiguousarray(np.asarray(w_self, dtype=np.float16).T)
    wneiT = np.ascontiguousarray(np.asarray(w_nei, dtype=np.float16).T)
    b_col = np.ascontiguousarray(np.asarray(b_self, dtype=np.float32).reshape(D, 1))

    core_of_dst = dst // NPC
    in_maps = []
    for cc in range(N_CORES):
        m = core_of_dst == cc
        counts = np.zeros((N_PAD, NPC), dtype=np.uint8)
        np.add.at(counts, (src[m], dst[m] - cc * NPC), 1)
        smat = (
            lut[np.minimum(counts, 63)]
            .reshape(NCH * 2, 128, NPC)
            .transpose(1, 0, 2)
            .reshape(128, NCH * 2 * NPC)
        )
        in_maps.append(
            {
                "h8": h8i,
                "smat": np.ascontiguousarray(smat).view(ml_dtypes.float8_e4m3),
                "hT": np.ascontiguousarray(hT16[:, cc * NPC : (cc + 1) * NPC]),
                "R": np.ascontiguousarray(R16[:, cc * NPC : (cc + 1) * NPC]),
                "wselfT": wselfT,
                "wneiT": wneiT,
                "bself": b_col,
            }
        )

    if "v41" not in _prog_cache:
        _prog_cache["v41"] = _build_program41()
    nc = _prog_cache["v41"]

    trace = bool(int(os.environ.get("GCN_TRACE", "0")))
    res = run_bass_kernel_spmd(nc, in_maps, core_ids=list(range(N_CORES)), trace=trace)
    kernel.last_results = res

    outT = np.concatenate([r["outT"] for r in res.results], axis=1)
    return np.ascontiguousarray(outT[:, :N_NODES].T).astype(np.float32)


# revision 9
# speedup vs baseline: 3.2325x; 3.2325x over previous
"""GCN v4.1: gather-free dense scatter in fp8 with DoubleRow matmuls.

Each core owns 2560 dst slots (identity slot map). The aggregation is
agg = hT @ A where A is the [20480 src, 2560 dst] edge-count matrix,
streamed from DRAM as fp8e4m3 (counts are small ints -> exact) in 80
src-pair-chunks of [128, 2, 2560]. h streams as fp8 [128, 2, 128] lhsT
pair-chunks (host pre-interleaved, no gather). DoubleRow perf mode
contracts 256 srcs per matmul at 0.5 cyc/col. Per-dst 1/deg scaling
happens on-device (DVE multiply by a host-replicated recip row) so fp8
carries no rounding error on the scatter weights.
"""

import numpy as np

N_NODES = 20000
D = 128
N_CORES = 8
N_PAD = 20480
NPC = N_PAD // N_CORES             # 2560 dst slots per core
NCH = N_PAD // 256                 # 80 src pair-chunks
TILE2 = 512
TPT = NPC // TILE2                 # 5 epilogue tiles per core

_prog_cache = {}


def _build_program41():
    import concourse.mybir as mybir
    from concourse import bacc
    from concourse.tile import TileContext

    dt = mybir.dt
    nc = bacc.Bacc()

    h8 = nc.declare_dram_parameter("h8", [128, NCH * 256], dt.float8e4, isOutput=False)
    smat = nc.declare_dram_parameter(
        "smat", [128, NCH * 2 * NPC], dt.float8e4, isOutput=False
    )
    hT = nc.declare_dram_parameter("hT", [D, NPC], dt.float16, isOutput=False)
    R = nc.declare_dram_parameter("R", [D, NPC], dt.float16, isOutput=False)
    wselfT = nc.declare_dram_parameter("wselfT", [D, D], dt.float16, isOutput=False)
    wneiT = nc.declare_dram_parameter("wneiT", [D, D], dt.float16, isOutput=False)
    bself = nc.declare_dram_parameter("bself", [D, 1], dt.float32, isOutput=False)
    outT = nc.declare_dram_parameter("outT", [D, NPC], dt.float16, isOutput=True)

    with (
        TileContext(nc) as tc,
        tc.tile_pool(name="const", bufs=1) as cpool,
        tc.tile_pool(name="hch", bufs=8) as hpool,
        tc.tile_pool(name="sch", bufs=8) as spool,
        tc.tile_pool(name="agg", bufs=2) as apool,
        tc.tile_pool(name="res", bufs=2) as opool,
        tc.tile_pool(name="pagg", bufs=1, space="PSUM") as pagg,
        tc.tile_pool(name="pout", bufs=2, space="PSUM") as pout,
    ):
        # resident h8: all lhsT pair-chunks, one 2.6MB preload
        h8_sb = cpool.tile([128, NCH * 256], dt.float8e4)
        nc.sync.dma_start(out=h8_sb[:], in_=h8[:])

        pa = pagg.tile([128, NPC], dt.float32)
        for c in range(NCH):
            sch = spool.tile([128, 2 * NPC], dt.float8e4)
            nc.sync.dma_start(
                out=sch[:], in_=smat[:, c * 2 * NPC : (c + 1) * 2 * NPC]
            )
            if c == 7:
                # epilogue-only consts: issue after the stream is rolling
                hT_sb = cpool.tile([D, NPC], dt.float16)
                nc.sync.dma_start(out=hT_sb[:], in_=hT[:])
                R_sb = cpool.tile([D, NPC], dt.float16)
                nc.sync.dma_start(out=R_sb[:], in_=R[:])
                wselfT_sb = cpool.tile([D, D], dt.float16)
                nc.sync.dma_start(out=wselfT_sb[:], in_=wselfT[:])
                wneiT_sb = cpool.tile([D, D], dt.float16)
                nc.sync.dma_start(out=wneiT_sb[:], in_=wneiT[:])
                bself_sb = cpool.tile([D, 1], dt.float32)
                nc.sync.dma_start(out=bself_sb[:], in_=bself[:])
            lhsT = h8_sb[:, c * 256 : (c + 1) * 256].rearrange(
                "s (j f) -> s j f", f=128
            )
            rhs3 = sch[:].rearrange("p (j n) -> p j n", j=2)
            for k in range(TPT):
                nc.tensor.matmul(
                    out=pa[:, k * TILE2 : (k + 1) * TILE2],
                    lhsT=lhsT,
                    rhs=rhs3[:, :, k * TILE2 : (k + 1) * TILE2],
                    perf_mode=mybir.MatmulPerfMode.DoubleRow,
                    start=(c == 0),
                    stop=(c == NCH - 1),
                )

        for k in range(TPT):
            sl = slice(k * TILE2, (k + 1) * TILE2)
            aggT = apool.tile([128, TILE2], dt.float16)
            nc.vector.tensor_mul(out=aggT[:], in0=pa[:, sl], in1=R_sb[:, sl])
            po = pout.tile([128, TILE2], dt.float32)
            nc.tensor.matmul(
                out=po[:], lhsT=wselfT_sb[:], rhs=hT_sb[:, sl], start=True, stop=False
            )
            nc.tensor.matmul(
                out=po[:], lhsT=wneiT_sb[:], rhs=aggT[:], start=False, stop=True
            )
            o = opool.tile([128, TILE2], dt.float16)
            nc.scalar.activation(
                out=o[:],
                in_=po[:],
                func=mybir.ActivationFunctionType.Relu,
                bias=bself_sb[:, :1],
            )
            nc.sync.dma_start(out=outT[:, sl], in_=o[:])

    nc.compile()
    return nc


def kernel(h, edge_index, deg, w_self, b_self, w_nei):
    import os

    import ml_dtypes
    from concourse.bass_utils import run_bass_kernel_spmd

    h = np.asarray(h, dtype=np.float32)
    deg = np.asarray(deg, dtype=np.float32)
    src = np.asarray(edge_index[0], dtype=np.int64)
    dst = np.asarray(edge_index[1], dtype=np.int64)

    # fp8 byte LUT for small integer counts (exact in e4m3)
    lut = np.arange(64, dtype=np.float32).astype(ml_dtypes.float8_e4m3).view(np.uint8)

    hpad = np.zeros((N_PAD, D), dtype=np.float32)
    hpad[:N_NODES] = h
    h8 = hpad.astype(ml_dtypes.float8_e4m3)
    # resident DoubleRow lhsT layout: [partition s, (pair c, ktile j, feat)]
    h8i = np.ascontiguousarray(
        h8.reshape(NCH, 2, 128, D).transpose(2, 0, 1, 3).reshape(128, NCH * 256)
    )
    hT16 = np.ascontiguousarray(hpad.astype(np.float16).T)  # [128, 20480]
    recip = np.ones(N_PAD, dtype=np.float32)
    recip[:N_NODES] = 1.0 / np.maximum(deg, 1.0)
    R16 = np.ascontiguousarray(
        np.broadcast_to(recip.astype(np.float16)[None, :], (D, N_PAD))
    )
    wselfT = np.ascontiguousarray(np.asarray(w_self, dtype=np.float16).T)
    wneiT = np.ascontiguousarray(np.asarray(w_nei, dtype=np.float16).T)
    b_col = np.ascontiguousarray(np.asarray(b_self, dtype=np.float32).reshape(D, 1))

    core_of_dst = dst // NPC
    in_maps = []
    for cc in range(N_CORES):
        m = core_of_dst == cc
        counts = np.zeros((N_PAD, NPC), dtype=np.uint8)
        np.add.at(counts, (src[m], dst[m] - cc * NPC), 1)
        smat = (
            lut[np.minimum(counts, 63)]
            .reshape(NCH * 2, 128, NPC)
            .transpose(1, 0, 2)
            .reshape(128, NCH * 2 * NPC)
        )
        in_maps.append(
            {
                "h8": h8i,
                "smat": np.ascontiguousarray(smat).view(ml_dtypes.float8_e4m3),
                "hT": np.ascontiguousarray(hT16[:, cc * NPC : (cc + 1) * NPC]),
                "R": np.ascontiguousarray(R16[:, cc * NPC : (cc + 1) * NPC]),
                "wselfT": wselfT,
                "wneiT": wneiT,
                "bself": b_col,
            }
        )

    if "v41" not in _prog_cache:
        _prog_cache["v41"] = _build_program41()
    nc = _prog_cache["v41"]

    trace = bool(int(os.environ.get("GCN_TRACE", "0")))
    res = run_bass_kernel_spmd(nc, in_maps, core_ids=list(range(N_CORES)), trace=trace)
    kernel.last_results = res

    outT = np.concatenate([r["outT"] for r in res.results], axis=1)
    return np.ascontiguousarray(outT[:, :N_NODES].T).astype(np.float32)


# revision 14
# speedup vs baseline: 3.3147x; 1.0254x over previous
"""GCN v4.1: gather-free dense scatter in fp8 with DoubleRow matmuls.

Each core owns 2560 dst slots (identity slot map). The aggregation is
agg = hT @ A where A is the [20480 src, 2560 dst] edge-count matrix,
streamed from DRAM as fp8e4m3 (counts are small ints -> exact) in 80
src-pair-chunks of [128, 2, 2560]. h streams as fp8 [128, 2, 128] lhsT
pair-chunks (host pre-interleaved, no gather). DoubleRow perf mode
contracts 256 srcs per matmul at 0.5 cyc/col. Per-dst 1/deg scaling
happens on-device (DVE multiply by a host-replicated recip row) so fp8
carries no rounding error on the scatter weights.
"""

import numpy as np

N_NODES = 20000
D = 128
N_CORES = 8
N_PAD = 20480
NPC = N_PAD // N_CORES             # 2560 dst slots per core
N_SRC = 20224                      # src rows padded to 79*256
NCH = N_SRC // 256                 # 79 src pair-chunks
TILE2 = 512
TPT = NPC // TILE2                 # 5 epilogue tiles per core

_prog_cache = {}


def _build_program41():
    import concourse.mybir as mybir
    from concourse import bacc
    from concourse.tile import TileContext

    dt = mybir.dt
    nc = bacc.Bacc()

    h8 = nc.declare_dram_parameter("h8", [128, NCH * 256], dt.float8e4, isOutput=False)
    smat = nc.declare_dram_parameter(
        "smat", [128, NCH * 2 * NPC], dt.float8e4, isOutput=False
    )
    hT = nc.declare_dram_parameter("hT", [D, NPC], dt.float16, isOutput=False)
    R = nc.declare_dram_parameter("R", [D, NPC], dt.float16, isOutput=False)
    wselfT = nc.declare_dram_parameter("wselfT", [D, D], dt.float16, isOutput=False)
    wneiT = nc.declare_dram_parameter("wneiT", [D, D], dt.float16, isOutput=False)
    bself = nc.declare_dram_parameter("bself", [D, 1], dt.float32, isOutput=False)
    outT = nc.declare_dram_parameter("outT", [D, NPC], dt.float16, isOutput=True)

    with (
        TileContext(nc) as tc,
        tc.tile_pool(name="const", bufs=1) as cpool,
        tc.tile_pool(name="hch", bufs=8) as hpool,
        tc.tile_pool(name="sch", bufs=12) as spool,
        tc.tile_pool(name="agg", bufs=2) as apool,
        tc.tile_pool(name="res", bufs=2) as opool,
        tc.tile_pool(name="pagg", bufs=1, space="PSUM") as pagg,
        tc.tile_pool(name="pout", bufs=2, space="PSUM") as pout,
    ):
        h8_sb = cpool.tile([128, NCH * 256], dt.float8e4)
        pa = pagg.tile([128, NPC], dt.float32)
        for c in range(NCH):
            sch = spool.tile([128, 2 * NPC], dt.float8e4)
            nc.sync.dma_start(
                out=sch[:], in_=smat[:, c * 2 * NPC : (c + 1) * 2 * NPC]
            )
            if c == 0:
                # resident h8: all lhsT pair-chunks, one 2.6MB preload
                # (must precede the first matmul in program order)
                nc.sync.dma_start(out=h8_sb[:], in_=h8[:])
            if c == 12:
                # epilogue-only consts: issue after the stream is rolling
                hT_sb = cpool.tile([D, NPC], dt.float16)
                nc.sync.dma_start(out=hT_sb[:], in_=hT[:])
                R_sb = cpool.tile([D, NPC], dt.float16)
                nc.sync.dma_start(out=R_sb[:], in_=R[:])
                wselfT_sb = cpool.tile([D, D], dt.float16)
                nc.sync.dma_start(out=wselfT_sb[:], in_=wselfT[:])
                wneiT_sb = cpool.tile([D, D], dt.float16)
                nc.sync.dma_start(out=wneiT_sb[:], in_=wneiT[:])
                bself_sb = cpool.tile([D, 1], dt.float32)
                nc.sync.dma_start(out=bself_sb[:], in_=bself[:])
            lhsT = h8_sb[:, c * 256 : (c + 1) * 256].rearrange(
                "s (j f) -> s j f", f=128
            )
            rhs3 = sch[:].rearrange("p (j n) -> p j n", j=2)
            for k in range(TPT):
                nc.tensor.matmul(
                    out=pa[:, k * TILE2 : (k + 1) * TILE2],
                    lhsT=lhsT,
                    rhs=rhs3[:, :, k * TILE2 : (k + 1) * TILE2],
                    perf_mode=mybir.MatmulPerfMode.DoubleRow,
                    start=(c == 0),
                    stop=(c == NCH - 1),
                )

        for k in range(TPT):
            sl = slice(k * TILE2, (k + 1) * TILE2)
            aggT = apool.tile([128, TILE2], dt.float16)
            nc.vector.tensor_mul(out=aggT[:], in0=pa[:, sl], in1=R_sb[:, sl])
            po = pout.tile([128, TILE2], dt.float32)
            nc.tensor.matmul(
                out=po[:], lhsT=wselfT_sb[:], rhs=hT_sb[:, sl], start=True, stop=False
            )
            nc.tensor.matmul(
                out=po[:], lhsT=wneiT_sb[:], rhs=aggT[:], start=False, stop=True
            )
            o = opool.tile([128, TILE2], dt.float16)
            nc.scalar.activation(
                out=o[:],
                in_=po[:],
                func=mybir.ActivationFunctionType.Relu,
                bias=bself_sb[:, :1],
            )
            nc.sync.dma_start(out=outT[:, sl], in_=o[:])

    nc.compile()
    return nc


def kernel(h, edge_index, deg, w_self, b_self, w_nei):
    import os

    import ml_dtypes
    from concourse.bass_utils import run_bass_kernel_spmd

    h = np.asarray(h, dtype=np.float32)
    deg = np.asarray(deg, dtype=np.float32)
    src = np.asarray(edge_index[0], dtype=np.int64)
    dst = np.asarray(edge_index[1], dtype=np.int64)

    # fp8 byte LUT for small integer counts (exact in e4m3)
    lut = np.arange(64, dtype=np.float32).astype(ml_dtypes.float8_e4m3).view(np.uint8)

    hsrc = np.zeros((N_SRC, D), dtype=np.float32)
    hsrc[:N_NODES] = h
    h8 = hsrc.astype(ml_dtypes.float8_e4m3)
    # resident DoubleRow lhsT layout: [partition s, (pair c, ktile j, feat)]
    h8i = np.ascontiguousarray(
        h8.reshape(NCH, 2, 128, D).transpose(2, 0, 1, 3).reshape(128, NCH * 256)
    )
    hpad = np.zeros((N_PAD, D), dtype=np.float32)
    hpad[:N_NODES] = h
    hT16 = np.ascontiguousarray(hpad.astype(np.float16).T)  # [128, 20480]
    recip = np.ones(N_PAD, dtype=np.float32)
    recip[:N_NODES] = 1.0 / np.maximum(deg, 1.0)
    R16 = np.ascontiguousarray(
        np.broadcast_to(recip.astype(np.float16)[None, :], (D, N_PAD))
    )
    wselfT = np.ascontiguousarray(np.asarray(w_self, dtype=np.float16).T)
    wneiT = np.ascontiguousarray(np.asarray(w_nei, dtype=np.float16).T)
    b_col = np.ascontiguousarray(np.asarray(b_self, dtype=np.float32).reshape(D, 1))

    core_of_dst = dst // NPC
    in_maps = []
    for cc in range(N_CORES):
        m = core_of_dst == cc
        counts = np.zeros((N_SRC, NPC), dtype=np.uint8)
        np.add.at(counts, (src[m], dst[m] - cc * NPC), 1)
        smat = (
            lut[np.minimum(counts, 63)]
            .reshape(NCH * 2, 128, NPC)
            .transpose(1, 0, 2)
            .reshape(128, NCH * 2 * NPC)
        )
        in_maps.append(
            {
                "h8": h8i,
                "smat": np.ascontiguousarray(smat).view(ml_dtypes.float8_e4m3),
                "hT": np.ascontiguousarray(hT16[:, cc * NPC : (cc + 1) * NPC]),
                "R": np.ascontiguousarray(R16[:, cc * NPC : (cc + 1) * NPC]),
                "wselfT": wselfT,
                "wneiT": wneiT,
                "bself": b_col,
            }
        )

    if "v41" not in _prog_cache:
        _prog_cache["v41"] = _build_program41()
    nc = _prog_cache["v41"]

    trace = bool(int(os.environ.get("GCN_TRACE", "0")))
    res = run_bass_kernel_spmd(nc, in_maps, core_ids=list(range(N_CORES)), trace=trace)
    kernel.last_results = res

    outT = np.concatenate([r["outT"] for r in res.results], axis=1)
    return np.ascontiguousarray(outT[:, :N_NODES].T).astype(np.float32)


# revision 16
# speedup vs baseline: 3.4103x; 1.0288x over previous
"""GCN v4.4: gather-free dense scatter in fp8 with DoubleRow matmuls.

Each core owns 2560 dst slots (identity slot map). The aggregation is
agg = hT @ A where A is the [20224 src, 2560 dst] edge-count matrix,
streamed from DRAM as fp8e4m3 (counts are small ints -> exact) in 40
quad-chunk DMAs of ~1.28MB. h is a 2.6MB SBUF-resident fp8 buffer in
DoubleRow-interleaved lhsT layout (no gather anywhere). DoubleRow perf
mode contracts 256 srcs per matmul at 0.5 cyc/col into one wide
[128, 2560] PSUM accumulator. Per-dst 1/deg scaling happens on-device
(gpsimd partition_broadcast of a [1,2560] recip row + DVE multiply) so
fp8 carries no rounding error on the scatter weights. The whole kernel
is DMA-bound at ~56MB/core; deep prefetch keeps the 16 queues >95%
busy mid-stream.
"""

import numpy as np

N_NODES = 20000
D = 128
N_CORES = 8
N_PAD = 20480
NPC = N_PAD // N_CORES             # 2560 dst slots per core
N_SRC = 20224                      # src rows padded to 79*256
NCH = N_SRC // 256                 # 79 src pair-chunks
TILE2 = 512
TPT = NPC // TILE2                 # 5 epilogue tiles per core

_prog_cache = {}


def _build_program41():
    import concourse.mybir as mybir
    from concourse import bacc
    from concourse.tile import TileContext

    dt = mybir.dt
    nc = bacc.Bacc()

    h8 = nc.declare_dram_parameter("h8", [128, NCH * 256], dt.float8e4, isOutput=False)
    smat = nc.declare_dram_parameter(
        "smat", [128, NCH * 2 * NPC], dt.float8e4, isOutput=False
    )
    hT = nc.declare_dram_parameter("hT", [D, NPC], dt.float16, isOutput=False)
    rrow = nc.declare_dram_parameter("rrow", [1, NPC], dt.float32, isOutput=False)
    wselfT = nc.declare_dram_parameter("wselfT", [D, D], dt.float16, isOutput=False)
    wneiT = nc.declare_dram_parameter("wneiT", [D, D], dt.float16, isOutput=False)
    bself = nc.declare_dram_parameter("bself", [D, 1], dt.float32, isOutput=False)
    outT = nc.declare_dram_parameter("outT", [D, NPC], dt.float16, isOutput=True)

    with (
        TileContext(nc) as tc,
        tc.tile_pool(name="const", bufs=1) as cpool,
        tc.tile_pool(name="hch", bufs=8) as hpool,
        tc.tile_pool(name="sch", bufs=6) as spool,
        tc.tile_pool(name="agg", bufs=2) as apool,
        tc.tile_pool(name="res", bufs=2) as opool,
        tc.tile_pool(name="pagg", bufs=1, space="PSUM") as pagg,
        tc.tile_pool(name="pout", bufs=2, space="PSUM") as pout,
    ):
        h8_sb = cpool.tile([128, NCH * 256], dt.float8e4)
        pa = pagg.tile([128, NPC], dt.float32)
        NQ = NCH // 2  # quad-chunk DMAs: 2 pair-chunks each
        assert NQ * 2 == NCH - 1
        for q in range(NQ + 1):
            npair = 2 if q < NQ else 1
            sch = spool.tile([128, 2 * npair * NPC], dt.float8e4, tag=f"s{npair}")
            nc.sync.dma_start(
                out=sch[:],
                in_=smat[:, q * 4 * NPC : q * 4 * NPC + 2 * npair * NPC],
            )
            if q == 0:
                # resident h8: all lhsT pair-chunks, one 2.6MB preload
                # (must precede the first matmul in program order)
                nc.sync.dma_start(out=h8_sb[:], in_=h8[:])
            if q == 6:
                # epilogue-only consts: issue after the stream is rolling
                hT_sb = cpool.tile([D, NPC], dt.float16)
                nc.sync.dma_start(out=hT_sb[:], in_=hT[:])
                rrow_sb = cpool.tile([1, NPC], dt.float32)
                nc.sync.dma_start(out=rrow_sb[:], in_=rrow[:])
                R_sb = cpool.tile([D, NPC], dt.float32)
                nc.gpsimd.partition_broadcast(R_sb[:], rrow_sb[:])
                wselfT_sb = cpool.tile([D, D], dt.float16)
                nc.sync.dma_start(out=wselfT_sb[:], in_=wselfT[:])
                wneiT_sb = cpool.tile([D, D], dt.float16)
                nc.sync.dma_start(out=wneiT_sb[:], in_=wneiT[:])
                bself_sb = cpool.tile([D, 1], dt.float32)
                nc.sync.dma_start(out=bself_sb[:], in_=bself[:])
            rhs4 = sch[:].rearrange("p (j n) -> p j n", n=NPC)
            for jp in range(npair):
                c = 2 * q + jp
                lhsT = h8_sb[:, c * 256 : (c + 1) * 256].rearrange(
                    "s (j f) -> s j f", f=128
                )
                for k in range(TPT):
                    nc.tensor.matmul(
                        out=pa[:, k * TILE2 : (k + 1) * TILE2],
                        lhsT=lhsT,
                        rhs=rhs4[
                            :,
                            2 * jp : 2 * jp + 2,
                            k * TILE2 : (k + 1) * TILE2,
                        ],
                        perf_mode=mybir.MatmulPerfMode.DoubleRow,
                        start=(c == 0),
                        stop=(c == NCH - 1),
                    )

        for k in range(TPT):
            sl = slice(k * TILE2, (k + 1) * TILE2)
            aggT = apool.tile([128, TILE2], dt.float16)
            nc.vector.tensor_mul(out=aggT[:], in0=pa[:, sl], in1=R_sb[:, sl])
            po = pout.tile([128, TILE2], dt.float32)
            nc.tensor.matmul(
                out=po[:], lhsT=wselfT_sb[:], rhs=hT_sb[:, sl], start=True, stop=False
            )
            nc.tensor.matmul(
                out=po[:], lhsT=wneiT_sb[:], rhs=aggT[:], start=False, stop=True
            )
            o = opool.tile([128, TILE2], dt.float16)
            nc.scalar.activation(
                out=o[:],
                in_=po[:],
                func=mybir.ActivationFunctionType.Relu,
                bias=bself_sb[:, :1],
            )
            nc.sync.dma_start(out=outT[:, sl], in_=o[:])

    nc.compile()
    return nc


def kernel(h, edge_index, deg, w_self, b_self, w_nei):
    import os

    import ml_dtypes
    from concourse.bass_utils import run_bass_kernel_spmd

    h = np.asarray(h, dtype=np.float32)
    deg = np.asarray(deg, dtype=np.float32)
    src = np.asarray(edge_index[0], dtype=np.int64)
    dst = np.asarray(edge_index[1], dtype=np.int64)

    # fp8 byte LUT for small integer counts (exact in e4m3)
    lut = np.arange(64, dtype=np.float32).astype(ml_dtypes.float8_e4m3).view(np.uint8)

    hsrc = np.zeros((N_SRC, D), dtype=np.float32)
    hsrc[:N_NODES] = h
    h8 = hsrc.astype(ml_dtypes.float8_e4m3)
    # resident DoubleRow lhsT layout: [partition s, (pair c, ktile j, feat)]
    h8i = np.ascontiguousarray(
        h8.reshape(NCH, 2, 128, D).transpose(2, 0, 1, 3).reshape(128, NCH * 256)
    )
    hpad = np.zeros((N_PAD, D), dtype=np.float32)
    hpad[:N_NODES] = h
    hT16 = np.ascontiguousarray(hpad.astype(np.float16).T)  # [128, 20480]
    recip = np.ones(N_PAD, dtype=np.float32)
    recip[:N_NODES] = 1.0 / np.maximum(deg, 1.0)

    wselfT = np.ascontiguousarray(np.asarray(w_self, dtype=np.float16).T)
    wneiT = np.ascontiguousarray(np.asarray(w_nei, dtype=np.float16).T)
    b_col = np.ascontiguousarray(np.asarray(b_self, dtype=np.float32).reshape(D, 1))

    core_of_dst = dst // NPC
    in_maps = []
    for cc in range(N_CORES):
        m = core_of_dst == cc
        counts = np.zeros((N_SRC, NPC), dtype=np.uint8)
        np.add.at(counts, (src[m], dst[m] - cc * NPC), 1)
        smat = (
            lut[np.minimum(counts, 63)]
            .reshape(NCH * 2, 128, NPC)
            .transpose(1, 0, 2)
            .reshape(128, NCH * 2 * NPC)
        )
        in_maps.append(
            {
                "h8": h8i,
                "smat": np.ascontiguousarray(smat).view(ml_dtypes.float8_e4m3),
                "hT": np.ascontiguousarray(hT16[:, cc * NPC : (cc + 1) * NPC]),
                "rrow": np.ascontiguousarray(
                    recip[cc * NPC : (cc + 1) * NPC].reshape(1, NPC)
                ),
                "wselfT": wselfT,
                "wneiT": wneiT,
                "bself": b_col,
            }
        )

    if "v41" not in _prog_cache:
        _prog_cache["v41"] = _build_program41()
    nc = _prog_cache["v41"]

    trace = bool(int(os.environ.get("GCN_TRACE", "0")))
    res = run_bass_kernel_spmd(nc, in_maps, core_ids=list(range(N_CORES)), trace=trace)
    kernel.last_results = res

    outT = np.concatenate([r["outT"] for r in res.results], axis=1)
    return np.ascontiguousarray(outT[:, :N_NODES].T).astype(np.float32)


# revision 17
# speedup vs baseline: 3.8321x; 1.1237x over previous
"""GCN v4.4: gather-free dense scatter in fp8 with DoubleRow matmuls.

Each core owns 2560 dst slots (identity slot map). The aggregation is
agg = hT @ A where A is the [20224 src, 2560 dst] edge-count matrix,
streamed from DRAM as fp8e4m3 (counts are small ints -> exact) in 40
quad-chunk DMAs of ~1.28MB. h is a 2.6MB SBUF-resident fp8 buffer in
DoubleRow-interleaved lhsT layout (no gather anywhere). DoubleRow perf
mode contracts 256 srcs per matmul at 0.5 cyc/col into one wide
[128, 2560] PSUM accumulator. Per-dst 1/deg scaling happens on-device
(gpsimd partition_broadcast of a [1,2560] recip row + DVE multiply) so
fp8 carries no rounding error on the scatter weights. The whole kernel
is DMA-bound at ~56MB/core; deep prefetch keeps the 16 queues >95%
busy mid-stream.
"""

import numpy as np

N_NODES = 20000
D = 128
N_CORES = 8
N_PAD = 20480
NPC = N_PAD // N_CORES             # 2560 dst slots per core
N_SRC = 20224                      # src rows padded to 79*256
NCH = N_SRC // 256                 # 79 src pair-chunks
TILE2 = 512
TPT = NPC // TILE2                 # 5 epilogue tiles per core

_prog_cache = {}


def _build_program41():
    import concourse.mybir as mybir
    from concourse import bacc
    from concourse.tile import TileContext

    dt = mybir.dt
    nc = bacc.Bacc()

    h8 = nc.declare_dram_parameter("h8", [128, NCH * 256], dt.float8e4, isOutput=False)
    smat = nc.declare_dram_parameter(
        "smat", [128, NCH * 2 * NPC], dt.float8e4, isOutput=False
    )
    hT = nc.declare_dram_parameter("hT", [D, NPC], dt.float16, isOutput=False)
    rrow = nc.declare_dram_parameter("rrow", [1, NPC], dt.float32, isOutput=False)
    wselfT = nc.declare_dram_parameter("wselfT", [D, D], dt.float16, isOutput=False)
    wneiT = nc.declare_dram_parameter("wneiT", [D, D], dt.float16, isOutput=False)
    bself = nc.declare_dram_parameter("bself", [D, 1], dt.float32, isOutput=False)
    outT = nc.declare_dram_parameter("outT", [D, NPC], dt.float16, isOutput=True)

    with (
        TileContext(nc) as tc,
        tc.tile_pool(name="const", bufs=1) as cpool,
        tc.tile_pool(name="sch", bufs=5) as spool,
        tc.tile_pool(name="agg", bufs=2) as apool,
        tc.tile_pool(name="res", bufs=2) as opool,
        tc.tile_pool(name="pagg", bufs=1, space="PSUM") as pagg,
        tc.tile_pool(name="pout", bufs=2, space="PSUM") as pout,
    ):
        h8_sb = cpool.tile([128, NCH * 256], dt.float8e4)
        pa = pagg.tile([128, NPC], dt.float32)
        NQ = NCH // 2  # quad-chunk DMAs: 2 pair-chunks each
        assert NQ * 2 == NCH - 1
        for q in range(NQ + 1):
            npair = 2 if q < NQ else 1
            sch = spool.tile([128, 2 * npair * NPC], dt.float8e4, tag=f"s{npair}")
            nc.sync.dma_start(
                out=sch[:],
                in_=smat[:, q * 4 * NPC : q * 4 * NPC + 2 * npair * NPC],
            )
            if q == 0:
                # resident h8: all lhsT pair-chunks, one 2.6MB preload
                # (must precede the first matmul in program order)
                nc.sync.dma_start(out=h8_sb[:], in_=h8[:])
            if q == 6:
                # epilogue-only consts: issue after the stream is rolling
                hT_sb = cpool.tile([D, NPC], dt.float16)
                nc.sync.dma_start(out=hT_sb[:], in_=hT[:])
                rrow_sb = cpool.tile([1, NPC], dt.float32)
                nc.sync.dma_start(out=rrow_sb[:], in_=rrow[:])
                R_sb = cpool.tile([D, NPC], dt.float32)
                nc.gpsimd.partition_broadcast(R_sb[:], rrow_sb[:])
                wselfT_sb = cpool.tile([D, D], dt.float16)
                nc.sync.dma_start(out=wselfT_sb[:], in_=wselfT[:])
                wneiT_sb = cpool.tile([D, D], dt.float16)
                nc.sync.dma_start(out=wneiT_sb[:], in_=wneiT[:])
                bself_sb = cpool.tile([D, 1], dt.float32)
                nc.sync.dma_start(out=bself_sb[:], in_=bself[:])
            rhs4 = sch[:].rearrange("p (j n) -> p j n", n=NPC)
            for jp in range(npair):
                c = 2 * q + jp
                lhsT = h8_sb[:, c * 256 : (c + 1) * 256].rearrange(
                    "s (j f) -> s j f", f=128
                )
                for k in range(TPT):
                    nc.tensor.matmul(
                        out=pa[:, k * TILE2 : (k + 1) * TILE2],
                        lhsT=lhsT,
                        rhs=rhs4[
                            :,
                            2 * jp : 2 * jp + 2,
                            k * TILE2 : (k + 1) * TILE2,
                        ],
                        perf_mode=mybir.MatmulPerfMode.DoubleRow,
                        start=(c == 0),
                        stop=(c == NCH - 1),
                    )

        for k in range(TPT):
            sl = slice(k * TILE2, (k + 1) * TILE2)
            aggT = apool.tile([128, TILE2], dt.float16)
            nc.vector.tensor_mul(out=aggT[:], in0=pa[:, sl], in1=R_sb[:, sl])
            po = pout.tile([128, TILE2], dt.float32)
            nc.tensor.matmul(
                out=po[:], lhsT=wselfT_sb[:], rhs=hT_sb[:, sl], start=True, stop=False
            )
            nc.tensor.matmul(
                out=po[:], lhsT=wneiT_sb[:], rhs=aggT[:], start=False, stop=True
            )
            o = opool.tile([128, TILE2], dt.float16)
            nc.scalar.activation(
                out=o[:],
                in_=po[:],
                func=mybir.ActivationFunctionType.Relu,
                bias=bself_sb[:, :1],
            )
            nc.sync.dma_start(out=outT[:, sl], in_=o[:])

    nc.compile()
    return nc


def kernel(h, edge_index, deg, w_self, b_self, w_nei):
    import os

    import ml_dtypes
    from concourse.bass_utils import run_bass_kernel_spmd

    h = np.asarray(h, dtype=np.float32)
    deg = np.asarray(deg, dtype=np.float32)
    src = np.asarray(edge_index[0], dtype=np.int64)
    dst = np.asarray(edge_index[1], dtype=np.int64)

    # fp8 byte LUT for small integer counts (exact in e4m3)
    lut = np.arange(64, dtype=np.float32).astype(ml_dtypes.float8_e4m3).view(np.uint8)

    hsrc = np.zeros((N_SRC, D), dtype=np.float32)
    hsrc[:N_NODES] = h
    h8 = hsrc.astype(ml_dtypes.float8_e4m3)
    # resident DoubleRow lhsT layout: [partition s, (pair c, ktile j, feat)]
    h8i = np.ascontiguousarray(
        h8.reshape(NCH, 2, 128, D).transpose(2, 0, 1, 3).reshape(128, NCH * 256)
    )
    hpad = np.zeros((N_PAD, D), dtype=np.float32)
    hpad[:N_NODES] = h
    hT16 = np.ascontiguousarray(hpad.astype(np.float16).T)  # [128, 20480]
    recip = np.ones(N_PAD, dtype=np.float32)
    recip[:N_NODES] = 1.0 / np.maximum(deg, 1.0)

    wselfT = np.ascontiguousarray(np.asarray(w_self, dtype=np.float16).T)
    wneiT = np.ascontiguousarray(np.asarray(w_nei, dtype=np.float16).T)
    b_col = np.ascontiguousarray(np.asarray(b_self, dtype=np.float32).reshape(D, 1))

    core_of_dst = dst // NPC
    in_maps = []
    for cc in range(N_CORES):
        m = core_of_dst == cc
        counts = np.zeros((N_SRC, NPC), dtype=np.uint8)
        np.add.at(counts, (src[m], dst[m] - cc * NPC), 1)
        smat = (
            lut[np.minimum(counts, 63)]
            .reshape(NCH * 2, 128, NPC)
            .transpose(1, 0, 2)
            .reshape(128, NCH * 2 * NPC)
        )
        in_maps.append(
            {
                "h8": h8i,
                "smat": np.ascontiguousarray(smat).view(ml_dtypes.float8_e4m3),
                "hT": np.ascontiguousarray(hT16[:, cc * NPC : (cc + 1) * NPC]),
                "rrow": np.ascontiguousarray(
                    recip[cc * NPC : (cc + 1) * NPC].reshape(1, NPC)
                ),
                "wselfT": wselfT,
                "wneiT": wneiT,
                "bself": b_col,
            }
        )

    if "v41" not in _prog_cache:
        _prog_cache["v41"] = _build_program41()
    nc = _prog_cache["v41"]

    trace = bool(int(os.environ.get("GCN_TRACE", "0")))
    res = run_bass_kernel_spmd(nc, in_maps, core_ids=list(range(N_CORES)), trace=trace)
    kernel.last_results = res

    outT = np.concatenate([r["outT"] for r in res.results], axis=1)
    return np.ascontiguousarray(outT[:, :N_NODES].T).astype(np.float32)
